# revision 25
# baseline (speedup 1.0000x reference)
"""Trainium2 Bass kernel for DecoderAttention (B=16, T=1024, D=1024, H=16).

Sharding: pure data-parallel over batch — 16 batch items / 8 cores = 2 per core.
No collectives. Each core runs the identical program on its 2 batch items.

The kernel is structured so that the softmax exp — the ScalarE throughput
floor (33.5M exps/core at 1 elem/lane/cycle ≈ 294 us) — hides almost all
projection PE work:

per batch item:
  A: hs [T,D] PE-transposed to hsT [D,T] (in column halves).
  V: V-projection (natural [t,j] layout + fused ones-column per head that
     makes the AV matmul emit the softmax denominator for free).
  C: per head-PAIR window: Q/K projection j-tiles for just this pair
     (transposed [j,t] layout), then per k-tile: QK^T logits (two heads
     row-packed on the PE's 64-row groups), exp on ScalarE (1/sqrt(64)
     folded into the activation affine), AV accumulate; then the softmax
     normalization: reciprocal (DVE) -> partition_broadcast (GpSimd) ->
     multiply (DVE) into attn_outT [d,t].
  D: out-projection from attn_outT (already the lhsT layout it needs).

All matmuls run in float32r (full PE rate at N>=256, ~1e-4..1e-3 rel err);
every f32r operand is produced by a rounding-capable engine (DVE/ACT) or is
an ExternalInput declared f32r.
"""

import os
import sys

import numpy as np

sys.path.insert(0, "/opt/trn_rl_repo")

import concourse.bass as bass  # noqa: E402
import concourse.mybir as mybir  # noqa: E402
import concourse.tile as tile  # noqa: E402
from concourse import bacc  # noqa: E402
from concourse.bass_utils import run_bass_kernel_spmd  # noqa: E402
from concourse.masks import make_identity  # noqa: E402

F32 = mybir.dt.float32
F32R = mybir.dt.float32r

B, T, D = 16, 1024, 1024
H, HD = 16, 64
N_CORES = 8
BL = B // N_CORES  # batch items per core
P = 128
CT = D // P  # contraction tiles (8)
TT = T // P  # token tiles (8)
NQ = 512  # matmul moving free dim
SCALE = 1.0 / np.sqrt(HD)

_last_results = None  # test.py reads this for the profile


def build_program():
    nc = bacc.Bacc(
        "TRN2", target_bir_lowering=False, debug=False, num_devices=N_CORES
    )

    hs = nc.dram_tensor("hidden_states", [BL, T, D], F32R, kind="ExternalInput")
    w_qkv = nc.dram_tensor("w_qkv", [D, 3 * D], F32R, kind="ExternalInput")
    b_qkv = nc.dram_tensor("b_qkv", [3 * D], F32, kind="ExternalInput")
    w_out = nc.dram_tensor("w_out", [D, D], F32R, kind="ExternalInput")
    b_out = nc.dram_tensor("b_out", [D], F32, kind="ExternalInput")
    out = nc.dram_tensor("out", [BL, T, D], F32, kind="ExternalOutput")

    Exp = mybir.ActivationFunctionType.Exp
    add = mybir.AluOpType.add
    mult = mybir.AluOpType.mult

    with tile.TileContext(nc) as tc:
        with (
            tc.tile_pool(name="consts", bufs=1) as consts,
            tc.tile_pool(name="main", bufs=1) as main,
            tc.tile_pool(name="pipe", bufs=2) as pipe,
            tc.tile_pool(name="psum", bufs=1, space="PSUM") as psum,
        ):
            # ---------------- constants ----------------
            identity = consts.tile([P, P], F32)
            make_identity(nc, identity)
            ones_ph = consts.tile([P, H, 1], F32)
            nc.gpsimd.memset(ones_ph, 1.0)
            identity_r = consts.tile([P, P], F32R)
            nc.vector.tensor_copy(identity_r, identity)
            # per-partition bias for QT/KT tiles: bq[p, jt] = b_qkv[jt*128+p]
            bq = consts.tile([P, 2 * CT], F32)
            nc.sync.dma_start(
                out=bq, in_=b_qkv.rearrange("(i p) -> p i", p=P)[:, 0 : 2 * CT]
            )
            # broadcast b_qkv V-slice and b_out along partitions via K=1 matmul
            bcast_bv = consts.tile([P, D], F32)
            bcast_bout = consts.tile([P, D], F32)

            def emit_bias_bcasts():
                bv_row = pipe.tile([1, D], F32, tag="exp", bufs=3, name="bv_row")
                nc.sync.dma_start(out=bv_row, in_=b_qkv[2 * D : 3 * D][None, :])
                bout_row = pipe.tile([1, D], F32, tag="exp", bufs=3,
                                     name="bout_row")
                nc.sync.dma_start(out=bout_row, in_=b_out[None, :])
                for dst, src in ((bcast_bv, bv_row), (bcast_bout, bout_row)):
                    nc.gpsimd.partition_broadcast(dst, src)

            for b in range(BL):
                # ------- A: hs -> hsT (two column halves, 4 hs tiles each) ---
                hsT = [
                    main.tile([P, T], F32R, tag=f"hsT{c}", name=f"hsT{b}_{c}")
                    for c in range(CT)
                ]
                for thalf in range(2):
                    for chalf in range(2):
                        hs_sb = []
                        for i in range(4):
                            t = thalf * 4 + i
                            h_t = main.tile([P, NQ], F32R, tag=f"hsl{i}",
                                            name=f"hs{b}_{t}_{chalf}")
                            # split each tile across both DMA queues so the
                            # first transposes start sooner
                            for piece in range(2):
                                psl = slice(piece * (NQ // 2),
                                            (piece + 1) * (NQ // 2))
                                dma_eng = (nc.sync, nc.gpsimd)[(i + piece) % 2]
                                dma_eng.dma_start(
                                    out=h_t[:, psl],
                                    in_=hs[b, t * P : (t + 1) * P,
                                           chalf * NQ : (chalf + 1) * NQ][:, psl],
                                )
                            hs_sb.append(h_t)
                        for cc in range(4):
                            c = chalf * 4 + cc
                            ps_tr = psum.tile([P, NQ], F32R, tag="p_big", bufs=2,
                                              name=f"ps_tr{b}_{thalf}_{c}")
                            for i in range(4):
                                nc.tensor.transpose(
                                    ps_tr[:, i * P : (i + 1) * P],
                                    hs_sb[i][:, cc * P : (cc + 1) * P],
                                    identity_r,
                                )
                            nc.vector.tensor_copy(
                                hsT[c][:, thalf * NQ : (thalf + 1) * NQ], ps_tr
                            )

                if b == 0:
                    emit_bias_bcasts()

                # ------- V-projection (wv resident for the batch) -----------
                wv_sb = []
                for c in range(CT):
                    wv_t = main.tile([P, D], F32R, tag=f"wv{c}", name=f"wv{b}_{c}")
                    nc.gpsimd.dma_start(
                        out=wv_t, in_=w_qkv[c * P : (c + 1) * P, 2 * D : 3 * D]
                    )
                    wv_sb.append(wv_t)
                V = []
                for t in range(TT):
                    v_t = main.tile([P, H * (HD + 1)], F32R, tag=f"v{t}",
                                    name=f"V{b}_{t}")
                    v3 = v_t.rearrange("p (h e) -> p h e", h=H)
                    nc.vector.tensor_copy(v3[:, :, HD : HD + 1], ones_ph)
                    for q in range(2):
                        sl = slice(q * NQ, (q + 1) * NQ)
                        hsl = slice(q * (H // 2), (q + 1) * (H // 2))
                        ps_v = psum.tile([P, NQ], F32, tag="p_av", bufs=4,
                                         name=f"ps_v{b}_{t}_{q}")
                        for c in range(CT):
                            nc.tensor.matmul(
                                ps_v,
                                hsT[c][:, t * P : (t + 1) * P],
                                wv_sb[c][:, sl],
                                start=(c == 0), stop=(c == CT - 1),
                            )
                        nc.vector.tensor_tensor(
                            out=v3[:, hsl, 0:HD],
                            in0=ps_v.rearrange("p (h e) -> p h e", h=H // 2),
                            in1=bcast_bv.rearrange("p (h e) -> p h e", h=H)[
                                :, hsl, :
                            ],
                            op=add,
                        )
                    V.append(v_t)

                # ------- C-window: per head pair ----------------------------
                attnT = [
                    main.tile([P, T], F32R, tag=f"wv{g}", name=f"attnT{b}_{g}")
                    for g in range(CT)
                ]
                def emit_norm_half(b, hp, av_h, i, q):
                    # per (head, q-half): recip (DVE) -> partition_broadcast
                    # (GpSimd) -> multiply (DVE). Emitted AFTER the next
                    # pair's Q/K bias-adds so the first QK^T matmul never
                    # queues behind the norm chain on DVE.
                    h = 2 * hp + i
                    g, r0 = hp, i * HD
                    sl = slice(q * NQ, (q + 1) * NQ)
                    av = av_h[2 * i + q]
                    recip_r = main.tile([1, NQ], F32R, tag=f"hsl{2 * i + q}",
                                        name=f"recip{b}_{h}_{q}")
                    with nc.allow_low_precision(
                        reason="softmax denom recip in f32r"
                    ):
                        nc.vector.reciprocal(recip_r, av[HD : HD + 1, :])
                    bc_sb = pipe.tile([HD, NQ], F32R, tag="bc_sb", bufs=4,
                                      name=f"bc{b}_{h}_{q}")
                    nc.gpsimd.partition_broadcast(bc_sb, recip_r)
                    nc.vector.tensor_tensor(
                        out=attnT[g][r0 : r0 + HD, sl],
                        in0=av[0:HD, :], in1=bc_sb, op=mult,
                    )

                def emit_norm(b, hp, av_h):
                    for i in range(2):
                        for q in range(2):
                            emit_norm_half(b, hp, av_h, i, q)

                prev_av = None
                for hp in range(H // 2):
                    pair_dst = []
                    for idx, (which, jt) in enumerate(
                        (("q", hp), ("k", CT + hp))
                    ):
                        wq_t = pipe.tile([P, CT, P], F32R, tag="wq",
                                         name=f"wq{b}_{jt}")
                        nc.sync.dma_start(
                            out=wq_t,
                            in_=w_qkv.rearrange("(c p) j -> p c j", p=P)[
                                :, :, jt * P : (jt + 1) * P
                            ],
                        )
                        ps_qk = psum.tile([P, T], F32, tag="p_big", bufs=2,
                                          name=f"ps_qk{b}_{jt}")
                        dst = main.tile([P, T], F32R, tag=f"{which}t{hp}",
                                        name=f"{which.upper()}T{b}_{hp}")
                        # q-half outer so each half's bias-add (DVE) overlaps
                        # the other half's matmuls instead of trailing them
                        for q in range(2):
                            sl = slice(q * NQ, (q + 1) * NQ)
                            for c in range(CT):
                                nc.tensor.matmul(
                                    ps_qk[:, sl], wq_t[:, c, :], hsT[c][:, sl],
                                    start=(c == 0), stop=(c == CT - 1),
                                )
                            nc.vector.tensor_scalar_add(
                                dst[:, sl], ps_qk[:, sl], bq[:, jt : jt + 1]
                            )
                        pair_dst.append(dst)
                    # normalize the PREVIOUS pair now that the bias-adds are
                    # queued: av(kt=0) of this pair frees its PSUM slot in
                    # recip->bcast->mult order, one q-half at a time
                    if prev_av is not None:
                        emit_norm(b, hp - 1, prev_av)
                    prev_av = None
                    QTg, KTg = pair_dst

                    ps_av = [
                        psum.tile([HD + 1, NQ], F32, tag="p_av", bufs=4,
                                  name=f"ps_av{b}_{2 * hp + i}_{q}")
                        for i in range(2)
                        for q in range(2)
                    ]
                    # kt loop software-pipelined by one step: QK+exp for kt
                    # are emitted before AV for kt-1, so the AV matmuls never
                    # wait on the exp that was just issued
                    def emit_qk_exp(kt):
                        expts = []
                        for i in range(2):  # head A (rows 0:64), head B (64:128)
                            r0 = i * HD
                            ps_l = psum.tile([P, T], F32, tag="p_big", bufs=2,
                                             name=f"ps_l{b}_{hp}_{kt}_{i}")
                            for q in range(2):
                                sl = slice(q * NQ, (q + 1) * NQ)
                                nc.tensor.matmul(
                                    ps_l[:, sl],
                                    KTg[r0 : r0 + HD, kt * P : (kt + 1) * P],
                                    QTg[r0 : r0 + HD, sl],
                                    start=True, stop=True,
                                )
                            expt = pipe.tile([P, T], F32R, tag="exp", bufs=3,
                                             name=f"exp{b}_{hp}_{kt}_{i}")
                            nc.scalar.activation(expt, ps_l, Exp,
                                                 scale=float(SCALE))
                            expts.append(expt)
                        return expts

                    def emit_av(kt, expts):
                        for i in range(2):
                            h = 2 * hp + i
                            for q in range(2):
                                sl = slice(q * NQ, (q + 1) * NQ)
                                nc.tensor.matmul(
                                    ps_av[2 * i + q],
                                    V[kt][:, h * (HD + 1) : (h + 1) * (HD + 1)],
                                    expts[i][:, sl],
                                    start=(kt == 0), stop=(kt == TT - 1),
                                )

                    pend = emit_qk_exp(0)
                    for kt in range(1, TT):
                        nxt = emit_qk_exp(kt)
                        emit_av(kt - 1, pend)
                        pend = nxt
                    emit_av(TT - 1, pend)
                    prev_av = ps_av
                emit_norm(b, H // 2 - 1, prev_av)

                # ------- D: out projection ----------------------------------
                wout_sb = []
                for dt in range(CT):
                    wo_t = main.tile([P, D], F32R, tag=f"hsT{dt}",
                                     name=f"wo{b}_{dt}")
                    nc.sync.dma_start(
                        out=wo_t, in_=w_out[dt * P : (dt + 1) * P, :]
                    )
                    wout_sb.append(wo_t)
                for t in range(TT):
                    ps_o = psum.tile([P, D], F32, tag="p_big", bufs=2,
                                     name=f"ps_o{b}_{t}")
                    for dt in range(CT):
                        for e in range(2):
                            sl = slice(e * NQ, (e + 1) * NQ)
                            nc.tensor.matmul(
                                ps_o[:, sl],
                                attnT[dt][:, t * P : (t + 1) * P],
                                wout_sb[dt][:, sl],
                                start=(dt == 0), stop=(dt == CT - 1),
                            )
                    o_t = pipe.tile([P, D], F32, tag="wq", name=f"o{b}_{t}")
                    nc.vector.tensor_tensor(
                        out=o_t, in0=ps_o, in1=bcast_bout, op=add
                    )
                    nc.gpsimd.dma_start(
                        out=out[b, t * P : (t + 1) * P, :], in_=o_t
                    )

    nc.compile()
    return nc


_nc_cache = None


def kernel(**inputs) -> np.ndarray:
    global _nc_cache, _last_results
    hs = np.ascontiguousarray(np.asarray(inputs["hidden_states"], dtype=np.float32))
    w_qkv = np.ascontiguousarray(np.asarray(inputs["w_qkv"], dtype=np.float32))
    b_qkv = np.ascontiguousarray(np.asarray(inputs["b_qkv"], dtype=np.float32))
    w_out = np.ascontiguousarray(np.asarray(inputs["w_out"], dtype=np.float32))
    b_out = np.ascontiguousarray(np.asarray(inputs["b_out"], dtype=np.float32))

    if _nc_cache is None:
        _nc_cache = build_program()
    nc = _nc_cache

    in_maps = [
        {
            "hidden_states": hs[c * BL : (c + 1) * BL],
            "w_qkv": w_qkv,
            "b_qkv": b_qkv,
            "w_out": w_out,
            "b_out": b_out,
        }
        for c in range(N_CORES)
    ]
    try:
        res = run_bass_kernel_spmd(
            nc,
            in_maps,
            list(range(N_CORES)),
            trace=bool(os.environ.get("BASS_TRACE")),
        )
    except ModuleNotFoundError:
        # trace path needs antenv.axon_hooks, absent in some containers —
        # retry with tracing suppressed
        prev = os.environ.get("BASS_NEVER_TRACE")
        os.environ["BASS_NEVER_TRACE"] = "1"
        try:
            res = run_bass_kernel_spmd(nc, in_maps, list(range(N_CORES)))
        finally:
            if prev is None:
                os.environ.pop("BASS_NEVER_TRACE", None)
            else:
                os.environ["BASS_NEVER_TRACE"] = prev
    _last_results = res
    return np.concatenate([res.results[c]["out"] for c in range(N_CORES)], axis=0)



# revision 28
# speedup vs baseline: 9.3355x; 9.3355x over previous
"""Trainium2 Bass kernel for DecoderAttention (B=16, T=1024, D=1024, H=16).

Sharding: pure data-parallel over batch — 16 batch items / 8 cores = 2 per core.
No collectives. Each core runs the identical program on its 2 batch items.

The kernel is structured so that the softmax exp — the ScalarE throughput
floor (33.5M exps/core at 1 elem/lane/cycle ≈ 294 us) — hides almost all
projection PE work:

per batch item:
  A: hs [T,D] PE-transposed to hsT [D,T] (in column halves).
  V: V-projection (natural [t,j] layout + fused ones-column per head that
     makes the AV matmul emit the softmax denominator for free).
  C: per head-PAIR window: Q/K projection j-tiles for just this pair
     (transposed [j,t] layout, q-half-outer so each half's bias-add
     overlaps the other half's matmuls), then per k-tile: QK^T logits
     (two heads row-packed on the PE's 64-row groups), exp on ScalarE
     (1/sqrt(64) folded into the activation affine), AV accumulate into
     per-(head, q-half) PSUM tiles; the softmax normalization
     (reciprocal on DVE -> partition_broadcast on GpSimd -> multiply on
     DVE into attn_outT [d,t]) runs per q-half and is emitted after the
     NEXT pair's bias-adds so the first QK^T never queues behind it.
  D: out-projection from attn_outT (already the lhsT layout it needs).

All broadcasts (softmax recip, biases) run on GpSimd partition_broadcast
— the PE does no K=1 broadcast matmuls.

All matmuls run in float32r (full PE rate at N>=256, ~1e-4..1e-3 rel err);
every f32r operand is produced by a rounding-capable engine (DVE/ACT) or is
an ExternalInput declared f32r.
"""

import os
import sys

import numpy as np

sys.path.insert(0, "/opt/trn_rl_repo")

import concourse.bass as bass  # noqa: E402
import concourse.mybir as mybir  # noqa: E402
import concourse.tile as tile  # noqa: E402
from concourse import bacc  # noqa: E402
from concourse.bass_utils import run_bass_kernel_spmd  # noqa: E402
from concourse.masks import make_identity  # noqa: E402

F32 = mybir.dt.float32
F32R = mybir.dt.float32r

B, T, D = 16, 1024, 1024
H, HD = 16, 64
N_CORES = 8
BL = B // N_CORES  # batch items per core
P = 128
CT = D // P  # contraction tiles (8)
TT = T // P  # token tiles (8)
NQ = 512  # matmul moving free dim
SCALE = 1.0 / np.sqrt(HD)

_last_results = None  # test.py reads this for the profile


def build_program():
    nc = bacc.Bacc(
        "TRN2", target_bir_lowering=False, debug=False, num_devices=N_CORES
    )

    hs = nc.dram_tensor("hidden_states", [BL, T, D], F32R, kind="ExternalInput")
    w_qkv = nc.dram_tensor("w_qkv", [D, 3 * D], F32R, kind="ExternalInput")
    b_qkv = nc.dram_tensor("b_qkv", [3 * D], F32, kind="ExternalInput")
    w_out = nc.dram_tensor("w_out", [D, D], F32R, kind="ExternalInput")
    b_out = nc.dram_tensor("b_out", [D], F32, kind="ExternalInput")
    out = nc.dram_tensor("out", [BL, T, D], F32, kind="ExternalOutput")

    Exp = mybir.ActivationFunctionType.Exp
    add = mybir.AluOpType.add
    mult = mybir.AluOpType.mult

    with tile.TileContext(nc) as tc:
        with (
            tc.tile_pool(name="consts", bufs=1) as consts,
            tc.tile_pool(name="main", bufs=1) as main,
            tc.tile_pool(name="pipe", bufs=2) as pipe,
            tc.tile_pool(name="psum", bufs=1, space="PSUM") as psum,
        ):
            # ---------------- constants ----------------
            identity = consts.tile([P, P], F32)
            make_identity(nc, identity)
            ones_ph = consts.tile([P, H, 1], F32)
            nc.gpsimd.memset(ones_ph, 1.0)
            identity_r = consts.tile([P, P], F32R)
            nc.vector.tensor_copy(identity_r, identity)
            # per-partition bias for QT/KT tiles: bq[p, jt] = b_qkv[jt*128+p]
            bq = consts.tile([P, 2 * CT], F32)
            nc.sync.dma_start(
                out=bq, in_=b_qkv.rearrange("(i p) -> p i", p=P)[:, 0 : 2 * CT]
            )
            # broadcast b_qkv V-slice and b_out along partitions via K=1 matmul
            bcast_bv = consts.tile([P, D], F32)
            bcast_bout = consts.tile([P, D], F32)

            def emit_bias_bcasts():
                bv_row = pipe.tile([1, D], F32, tag="exp", bufs=3, name="bv_row")
                nc.sync.dma_start(out=bv_row, in_=b_qkv[2 * D : 3 * D][None, :])
                bout_row = pipe.tile([1, D], F32, tag="exp", bufs=3,
                                     name="bout_row")
                nc.sync.dma_start(out=bout_row, in_=b_out[None, :])
                for dst, src in ((bcast_bv, bv_row), (bcast_bout, bout_row)):
                    nc.gpsimd.partition_broadcast(dst, src)

            for b in range(BL):
                # ------- A: hs -> hsT (two column halves, 4 hs tiles each) ---
                hsT = [
                    main.tile([P, T], F32R, tag=f"hsT{c}", name=f"hsT{b}_{c}")
                    for c in range(CT)
                ]
                for thalf in range(2):
                    for chalf in range(2):
                        hs_sb = []
                        for i in range(4):
                            t = thalf * 4 + i
                            h_t = main.tile([P, NQ], F32R, tag=f"hsl{i}",
                                            name=f"hs{b}_{t}_{chalf}")
                            # split each tile across both DMA queues so the
                            # first transposes start sooner
                            for piece in range(2):
                                psl = slice(piece * (NQ // 2),
                                            (piece + 1) * (NQ // 2))
                                dma_eng = (nc.sync, nc.gpsimd)[(i + piece) % 2]
                                dma_eng.dma_start(
                                    out=h_t[:, psl],
                                    in_=hs[b, t * P : (t + 1) * P,
                                           chalf * NQ : (chalf + 1) * NQ][:, psl],
                                )
                            hs_sb.append(h_t)
                        for cc in range(4):
                            c = chalf * 4 + cc
                            ps_tr = psum.tile([P, NQ], F32R, tag="p_big", bufs=2,
                                              name=f"ps_tr{b}_{thalf}_{c}")
                            for i in range(4):
                                nc.tensor.transpose(
                                    ps_tr[:, i * P : (i + 1) * P],
                                    hs_sb[i][:, cc * P : (cc + 1) * P],
                                    identity_r,
                                )
                            nc.vector.tensor_copy(
                                hsT[c][:, thalf * NQ : (thalf + 1) * NQ], ps_tr
                            )

                if b == 0:
                    emit_bias_bcasts()

                # ------- V-projection (wv resident for the batch) -----------
                wv_sb = []
                for c in range(CT):
                    wv_t = main.tile([P, D], F32R, tag=f"wv{c}", name=f"wv{b}_{c}")
                    nc.gpsimd.dma_start(
                        out=wv_t, in_=w_qkv[c * P : (c + 1) * P, 2 * D : 3 * D]
                    )
                    wv_sb.append(wv_t)
                V = []
                for t in range(TT):
                    v_t = main.tile([P, H * (HD + 1)], F32R, tag=f"v{t}",
                                    name=f"V{b}_{t}")
                    v3 = v_t.rearrange("p (h e) -> p h e", h=H)
                    nc.vector.tensor_copy(v3[:, :, HD : HD + 1], ones_ph)
                    for q in range(2):
                        sl = slice(q * NQ, (q + 1) * NQ)
                        hsl = slice(q * (H // 2), (q + 1) * (H // 2))
                        ps_v = psum.tile([P, NQ], F32, tag="p_av", bufs=4,
                                         name=f"ps_v{b}_{t}_{q}")
                        for c in range(CT):
                            nc.tensor.matmul(
                                ps_v,
                                hsT[c][:, t * P : (t + 1) * P],
                                wv_sb[c][:, sl],
                                start=(c == 0), stop=(c == CT - 1),
                            )
                        nc.vector.tensor_tensor(
                            out=v3[:, hsl, 0:HD],
                            in0=ps_v.rearrange("p (h e) -> p h e", h=H // 2),
                            in1=bcast_bv.rearrange("p (h e) -> p h e", h=H)[
                                :, hsl, :
                            ],
                            op=add,
                        )
                    V.append(v_t)

                # ------- C-window: per head pair ----------------------------
                attnT = [
                    main.tile([P, T], F32R, tag=f"wv{g}", name=f"attnT{b}_{g}")
                    for g in range(CT)
                ]
                def emit_norm_half(b, hp, av_h, i, q):
                    # per (head, q-half): recip (DVE) -> partition_broadcast
                    # (GpSimd) -> multiply (DVE). Emitted AFTER the next
                    # pair's Q/K bias-adds so the first QK^T matmul never
                    # queues behind the norm chain on DVE.
                    h = 2 * hp + i
                    g, r0 = hp, i * HD
                    sl = slice(q * NQ, (q + 1) * NQ)
                    av = av_h[2 * i + q]
                    recip_r = main.tile([1, NQ], F32R, tag=f"hsl{2 * i + q}",
                                        name=f"recip{b}_{h}_{q}")
                    with nc.allow_low_precision(
                        reason="softmax denom recip in f32r"
                    ):
                        nc.vector.reciprocal(recip_r, av[HD : HD + 1, :])
                    bc_sb = pipe.tile([HD, NQ], F32R, tag="bc_sb", bufs=4,
                                      name=f"bc{b}_{h}_{q}")
                    nc.gpsimd.partition_broadcast(bc_sb, recip_r)
                    nc.vector.tensor_tensor(
                        out=attnT[g][r0 : r0 + HD, sl],
                        in0=av[0:HD, :], in1=bc_sb, op=mult,
                    )

                def emit_norm(b, hp, av_h):
                    for i in range(2):
                        for q in range(2):
                            emit_norm_half(b, hp, av_h, i, q)

                prev_av = None
                for hp in range(H // 2):
                    pair_dst = []
                    for idx, (which, jt) in enumerate(
                        (("q", hp), ("k", CT + hp))
                    ):
                        wq_t = pipe.tile([P, CT, P], F32R, tag="wq",
                                         name=f"wq{b}_{jt}")
                        nc.sync.dma_start(
                            out=wq_t,
                            in_=w_qkv.rearrange("(c p) j -> p c j", p=P)[
                                :, :, jt * P : (jt + 1) * P
                            ],
                        )
                        ps_qk = psum.tile([P, T], F32, tag="p_big", bufs=2,
                                          name=f"ps_qk{b}_{jt}")
                        dst = main.tile([P, T], F32R, tag=f"{which}t{hp}",
                                        name=f"{which.upper()}T{b}_{hp}")
                        # q-half outer so each half's bias-add (DVE) overlaps
                        # the other half's matmuls instead of trailing them
                        for q in range(2):
                            sl = slice(q * NQ, (q + 1) * NQ)
                            for c in range(CT):
                                nc.tensor.matmul(
                                    ps_qk[:, sl], wq_t[:, c, :], hsT[c][:, sl],
                                    start=(c == 0), stop=(c == CT - 1),
                                )
                            nc.vector.tensor_scalar_add(
                                dst[:, sl], ps_qk[:, sl], bq[:, jt : jt + 1]
                            )
                        pair_dst.append(dst)
                    # normalize the PREVIOUS pair now that the bias-adds are
                    # queued: av(kt=0) of this pair frees its PSUM slot in
                    # recip->bcast->mult order, one q-half at a time
                    if prev_av is not None:
                        emit_norm(b, hp - 1, prev_av)
                    prev_av = None
                    QTg, KTg = pair_dst

                    ps_av = [
                        psum.tile([HD + 1, NQ], F32, tag="p_av", bufs=4,
                                  name=f"ps_av{b}_{2 * hp + i}_{q}")
                        for i in range(2)
                        for q in range(2)
                    ]
                    # kt loop software-pipelined by one step: QK+exp for kt
                    # are emitted before AV for kt-1, so the AV matmuls never
                    # wait on the exp that was just issued
                    def emit_qk_exp(kt):
                        expts = []
                        for i in range(2):  # head A (rows 0:64), head B (64:128)
                            r0 = i * HD
                            ps_l = psum.tile([P, T], F32, tag="p_big", bufs=2,
                                             name=f"ps_l{b}_{hp}_{kt}_{i}")
                            for q in range(2):
                                sl = slice(q * NQ, (q + 1) * NQ)
                                nc.tensor.matmul(
                                    ps_l[:, sl],
                                    KTg[r0 : r0 + HD, kt * P : (kt + 1) * P],
                                    QTg[r0 : r0 + HD, sl],
                                    start=True, stop=True,
                                )
                            expt = pipe.tile([P, T], F32R, tag="exp", bufs=3,
                                             name=f"exp{b}_{hp}_{kt}_{i}")
                            nc.scalar.activation(expt, ps_l, Exp,
                                                 scale=float(SCALE))
                            expts.append(expt)
                        return expts

                    def emit_av(kt, expts):
                        for i in range(2):
                            h = 2 * hp + i
                            for q in range(2):
                                sl = slice(q * NQ, (q + 1) * NQ)
                                nc.tensor.matmul(
                                    ps_av[2 * i + q],
                                    V[kt][:, h * (HD + 1) : (h + 1) * (HD + 1)],
                                    expts[i][:, sl],
                                    start=(kt == 0), stop=(kt == TT - 1),
                                )

                    pend = emit_qk_exp(0)
                    for kt in range(1, TT):
                        nxt = emit_qk_exp(kt)
                        emit_av(kt - 1, pend)
                        pend = nxt
                    emit_av(TT - 1, pend)
                    prev_av = ps_av
                emit_norm(b, H // 2 - 1, prev_av)

                # ------- D: out projection ----------------------------------
                wout_sb = []
                for dt in range(CT):
                    wo_t = main.tile([P, D], F32R, tag=f"hsT{dt}",
                                     name=f"wo{b}_{dt}")
                    nc.sync.dma_start(
                        out=wo_t, in_=w_out[dt * P : (dt + 1) * P, :]
                    )
                    wout_sb.append(wo_t)
                for t in range(TT):
                    ps_o = psum.tile([P, D], F32, tag="p_big", bufs=2,
                                     name=f"ps_o{b}_{t}")
                    for dt in range(CT):
                        for e in range(2):
                            sl = slice(e * NQ, (e + 1) * NQ)
                            nc.tensor.matmul(
                                ps_o[:, sl],
                                attnT[dt][:, t * P : (t + 1) * P],
                                wout_sb[dt][:, sl],
                                start=(dt == 0), stop=(dt == CT - 1),
                            )
                    o_t = pipe.tile([P, D], F32, tag="wq", name=f"o{b}_{t}")
                    nc.vector.tensor_tensor(
                        out=o_t, in0=ps_o, in1=bcast_bout, op=add
                    )
                    nc.gpsimd.dma_start(
                        out=out[b, t * P : (t + 1) * P, :], in_=o_t
                    )

    nc.compile()
    return nc


_nc_cache = None


def kernel(**inputs) -> np.ndarray:
    global _nc_cache, _last_results
    hs = np.ascontiguousarray(np.asarray(inputs["hidden_states"], dtype=np.float32))
    w_qkv = np.ascontiguousarray(np.asarray(inputs["w_qkv"], dtype=np.float32))
    b_qkv = np.ascontiguousarray(np.asarray(inputs["b_qkv"], dtype=np.float32))
    w_out = np.ascontiguousarray(np.asarray(inputs["w_out"], dtype=np.float32))
    b_out = np.ascontiguousarray(np.asarray(inputs["b_out"], dtype=np.float32))

    if _nc_cache is None:
        _nc_cache = build_program()
    nc = _nc_cache

    in_maps = [
        {
            "hidden_states": hs[c * BL : (c + 1) * BL],
            "w_qkv": w_qkv,
            "b_qkv": b_qkv,
            "w_out": w_out,
            "b_out": b_out,
        }
        for c in range(N_CORES)
    ]
    try:
        res = run_bass_kernel_spmd(
            nc,
            in_maps,
            list(range(N_CORES)),
            trace=bool(os.environ.get("BASS_TRACE")),
        )
    except ModuleNotFoundError:
        # trace path needs antenv.axon_hooks, absent in some containers —
        # retry with tracing suppressed
        prev = os.environ.get("BASS_NEVER_TRACE")
        os.environ["BASS_NEVER_TRACE"] = "1"
        try:
            res = run_bass_kernel_spmd(nc, in_maps, list(range(N_CORES)))
        finally:
            if prev is None:
                os.environ.pop("BASS_NEVER_TRACE", None)
            else:
                os.environ["BASS_NEVER_TRACE"] = prev
    _last_results = res
    return np.concatenate([res.results[c]["out"] for c in range(N_CORES)], axis=0)

